# revision 29
# baseline (speedup 1.0000x reference)
"""Multi-head self-attention with additive position bias, data-parallel across
8 TRN2 NeuronCores (one batch element per core).

Per core (batch b), everything is computed in a transposed layout so that no
on-device transposes are needed:
  - host supplies xT = x[b].T (fp16) and epos[h] = exp(pos[h].T / sqrt(D)) (fp16)
  - qT/kT    = W_{q,k}.T @ xT                    [cols, N]   (PE, fp16)
  - v        = xT.T @ W_v                        [N, cols]   (PE, fp16), stored
               with a literal 1.0 column after each head's 64 columns
  - scoresT  = kT_h(m-tile).T @ qT_h             [m, n]      (PE, head-pairs
               packed into row groups 0-63 / 64-127 of the systolic array)
  - estT     = exp(scoresT/sqrt(D)) * eposT      (ACT exp + DVE mul; the
               additive bias becomes a multiplicative factor after exp)
  - outT_h   = v_aug,h.T @ estT : [65, n] accumulated over m-tiles.  The
               stationary operand is the 65-column v_aug (cheap LDWEIGHTS,
               fully hidden under the 512-element streams), and partition 64
               receives the softmax denominators for free — no separate
               ones-vector matmuls.
  - normalization: denominators are reshaped across lanes via two tiny DMAs,
    inverted on DVE, DMA'd back to [1, N] rows, broadcast with one-row
    matmuls, and multiplied into ATTNT.  The broadcast is deferred into the
    NEXT pair's loop so the small-DMA latency chain never stalls the PE.
  - out      = attnT.T @ W_proj                  [N, C] fp32
"""

import numpy as np

N_CORES = 8
N = 1024
C = 768
H = 12
D = 64
E = D + 1  # 65: v columns + ones column per head
HP = H // 2  # head pairs
SCALE = 0.125  # 1/sqrt(D)

# ---------------------------------------------------------------------------
# walrus in this toolchain rejects instructions carrying more than one sync
# wait ("Too many sync wait commands").  Tile's semaphore pass can attach
# several (esp. the kernel-tail drain).  Spread surplus waits across InstNoOp
# instructions inserted immediately before the oversubscribed instruction in
# the same basic block / engine stream — semantically identical, since the
# engine sequencer performs the waits in stream order.
# ---------------------------------------------------------------------------


def _apply_tile_patch():
    from concourse import mybir
    from concourse.tile import TileContext
    from concourse.vector_clock import ScopedClock

    def _patched_drain_and_barrier(self, tick_clock, wait_clock):
        nc = self.nc
        drain_inst = nc.sync.drain()
        wait_clock.add_sem_waits(
            drain_inst.ins, ScopedClock({None: tick_clock.global_clock})
        )
        nc.all_engine_barrier()
        assert self.sems is not None
        popped = nc._tile_sem_poison_stack.pop()
        assert popped is self._sem_poison
        nc.clear_and_free_semaphores(list(self.sems.allocated().values()))
        nc.all_engine_barrier()

    TileContext._drain_and_barrier = _patched_drain_and_barrier


def _split_excess_waits(nc, max_waits=1):
    from concourse import mybir

    n_split = 0
    for f in nc.m.functions:
        for blk in f.blocks:
            insts = blk.instructions
            new_list = []
            changed = False
            for inst in insts:
                si = inst.sync_info
                waits = list(si.on_wait) if (si is not None and si.on_wait) else []
                eff_max = max_waits
                if len(waits) > eff_max:
                    step = max(eff_max, 1)
                    extra = waits if eff_max == 0 else waits[: len(waits) - eff_max]
                    keep = [] if eff_max == 0 else waits[len(waits) - eff_max :]
                    for i in range(0, len(extra), step):
                        nop = mybir.InstNoOp(
                            name=nc.get_next_instruction_name(),
                            engine=inst.engine,
                            ins=[],
                            outs=[],
                            sync_info=mybir.SyncInfo(
                                on_wait=extra[i : i + step], on_update=[]
                            ),
                        )
                        nc.register_instruction(nop, overwrite=True)
                        new_list.append(nop)
                        n_split += 1
                    inst.sync_info = mybir.SyncInfo(
                        on_wait=keep,
                        on_update=list(si.on_update) if si.on_update else [],
                    )
                    changed = True
                new_list.append(inst)
            if changed:
                blk.instructions = new_list
    return n_split


def build(has_bias):
    import concourse.bass as bass
    import concourse.mybir as mybir
    from concourse.tile import TileContext

    _apply_tile_patch()

    FP16 = mybir.dt.float16
    F32 = mybir.dt.float32
    EXP = mybir.ActivationFunctionType.Exp

    nc = bass.Bass()
    xt_ext = nc.declare_dram_parameter("xt", [C, N], FP16, isOutput=False)
    wqkv_ext = nc.declare_dram_parameter("wqkv", [C, 3 * C], FP16, isOutput=False)
    wproj_ext = nc.declare_dram_parameter("wproj", [C, C], FP16, isOutput=False)
    epos_ext = nc.declare_dram_parameter("epos", [H, N, N], FP16, isOutput=False)
    if has_bias:
        bqkv_ext = nc.declare_dram_parameter("bqkv", [1, 3 * C], FP16, isOutput=False)
        bproj_ext = nc.declare_dram_parameter("bproj", [1, C], FP16, isOutput=False)
    out_ext = nc.declare_dram_parameter("out", [N, C], F32, isOutput=True)

    KT = C // 128  # 6 contraction tiles
    NT = N // 128  # 8 n-tiles / m-tiles

    with TileContext(nc) as tc:
        with (
            tc.tile_pool(name="const", bufs=1) as const,
            tc.tile_pool(name="epp", bufs=4) as epp_pool,
            tc.tile_pool(name="est", bufs=16) as est_pool,
            tc.tile_pool(name="qkt", bufs=3) as qkt_pool,
            tc.tile_pool(name="sgp", bufs=1) as sgp_pool,
            tc.tile_pool(name="stg", bufs=2) as stg_pool,
            tc.tile_pool(name="s2p", bufs=2) as s2_pool,
            tc.tile_pool(name="invr", bufs=4) as invr_pool,
            tc.tile_pool(name="outsb", bufs=2) as outsb_pool,
            tc.tile_pool(name="ps", bufs=1, space="PSUM") as ps,
        ):
            def _ps_tile(shape, tag):
                return ps.tile(
                    shape, F32, tag=tag, bufs=2 if tag in ("bc", "sc") else 1,
                    name=f"ps_{tag}",
                )

            XT = const.tile([128, KT, N], FP16)
            WQKV = const.tile([128, KT, 3 * C], FP16)
            WPROJ = const.tile([128, KT, C], FP16)
            xt_r = xt_ext.rearrange("(t p) n -> p t n", p=128)
            wqkv_r = wqkv_ext.rearrange("(t p) n -> p t n", p=128)

            ONES1x64 = const.tile([1, 64], FP16)
            nc.vector.memset(ONES1x64[:], 1.0)
            # exp table-set preload (~2.7us) during the startup DMA window —
            # otherwise it lands in front of the first real exp
            WARM = const.tile([1, 512], FP16)
            nc.vector.memset(WARM[:], 0.0)
            DXO = const.tile([1, 16], FP16)
            nc.scalar.activation(DXO[:], WARM[0:1, 0:16], EXP, scale=1.0)

            # The critical chain to the first exp is: Wq|k(pair0) + x -> qkT
            # prelude -> scores.  W_qkv columns are HOST-permuted so each
            # pair's q|k block is a contiguous 256-col slab; pair 0's slab
            # loads first, then x (the prelude accumulates kt-ordered, so it
            # starts as soon as the first x chunk lands).  Wv on the scalar
            # queue in parallel (v-proj groups run inside iteration 0).
            # Pairs 2-5 + Wproj are deferred into the loop's epos stream.
            nc.sync.dma_start(out=WQKV[:, :, 0:256], in_=wqkv_r[:, :, 0:256])
            for k2 in range(3):
                nc.sync.dma_start(
                    out=XT[:, 2 * k2 : 2 * k2 + 2, :], in_=xt_r[:, 2 * k2 : 2 * k2 + 2, :]
                )
            # All bulk loads ride the ONE sync queue: same-queue DMAs drain
            # strictly FIFO, so ordering here IS the bandwidth priority
            # (cross-queue DMAs would round-robin and steal from x).  The
            # gpsimd queue is left empty for the tiny latency-critical norm
            # shuffles.
            nc.sync.dma_start(out=WQKV[:, :, 256:512], in_=wqkv_r[:, :, 256:512])
            for vh in range(2):
                nc.sync.dma_start(
                    out=WQKV[:, 3 * vh : 3 * vh + 3, 2 * C : 3 * C],
                    in_=wqkv_r[:, 3 * vh : 3 * vh + 3, 2 * C : 3 * C],
                )
            if has_bias:
                BQKV = const.tile([1, 3 * C], FP16)
                BPROJ = const.tile([1, C], FP16)
                ONESROW = const.tile([1, N], FP16)
                nc.sync.dma_start(out=BQKV[:], in_=bqkv_ext[:])
                nc.sync.dma_start(out=BPROJ[:], in_=bproj_ext[:])
                nc.vector.memset(ONESROW[:], 1.0)

            # PE warmup: the HAM clock gate starts at K=4/8 (1.2 GHz) and
            # un-throttles only after ~3.4us of sustained PE activity.  Burn
            # the x-DMA wait window with a few dummy matmuls (few enough not
            # to delay the prelude once x lands).
            for _ in range(4):
                WPS = _ps_tile([64, 512], "bc")
                nc.tensor.matmul(
                    WPS[:, 0:512], ONES1x64[:], WARM[0:1, :], start=True, stop=True
                )

            # v in [n, col] layout with a 1.0 column after each head's 64
            # columns: with v_aug as the matmul's stationary operand, output
            # partition 64 accumulates the softmax denominators for free.
            VN65 = const.tile([128, NT, H * E], FP16)
            for h in range(H):
                nc.vector.memset(VN65[:, :, h * E + D : h * E + E], 1.0)
            ATTNT = const.tile([128, KT, N], FP16)
            # softmax denominators parked at partition 64 (same partition the
            # attn@v matmuls write them to — engines cannot shift partitions)
            SGP = sgp_pool.tile([65, 2 * N], F32, tag="sgp")

            _vtags = ["sc", "ot", "bc", "bc"]

            # ---- V projection groups: v[n, vcol] = xT.T @ Wv (+ b_v) ----
            # Not a pre-loop phase: groups are scheduled INSIDE the pair loop
            # (iteration 0 has no attn@v work, so its PE slack absorbs the
            # vs=0 half; vs=1 rides iterations 1-2, well before pair 3 needs
            # it).  This moves ~15us of GEMM off the pre-exp critical path.
            def vproj_group(nt, vs, tag):
                pv = _ps_tile([128, 384], tag)
                dst = pv[:, 0:384]
                for kt in range(KT):
                    nc.tensor.matmul(
                        dst,
                        XT[:, kt, nt * 128 : (nt + 1) * 128],
                        WQKV[:, kt, 2 * C + vs * 384 : 2 * C + (vs + 1) * 384],
                        start=(kt == 0),
                        stop=(kt == KT - 1 and not has_bias),
                    )
                if has_bias:
                    nc.tensor.matmul(
                        dst,
                        ONESROW[0:1, nt * 128 : (nt + 1) * 128],
                        BQKV[0:1, 2 * C + vs * 384 : 2 * C + (vs + 1) * 384],
                        start=False,
                        stop=True,
                    )
                nc.vector.tensor_copy(
                    VN65[:, nt, vs * 6 * E : (vs + 1) * 6 * E].rearrange(
                        "p (h e) -> p h e", e=E
                    )[:, :, 0:D],
                    dst.rearrange("p (h d) -> p h d", d=D),
                )

            # (hp, mt) -> [(nt, vs, psum tag)]
            vp_sched = {}
            for mt in range(8):
                vp_sched[(0, mt)] = [(mt, 0, "ot")]
            for mt in range(4):
                vp_sched[(1, mt)] = [(mt, 1, "bc")]
                vp_sched[(2, mt)] = [(4 + mt, 1, "bc")]

            # ---- head-pair loop, software-pipelined one pair deep:
            # pair hp:   scores -> exp -> est     (ACT-bound phase)
            # pair hp-1: v_aug.T @ est            (dense PE work, fills gaps)
            # pair hp+1: qT/kT projection chunks  (always-ready PE filler)
            # pair hp-2: deferred normalize broadcast (hides sums-DMA latency)

            qkt_state = {}
            qkt_tiles = {}

            def qkt_half(pair, mt):
                # 24 qkT matmuls spread 4-per-mt over mts 0..5, so the final
                # cast lands two mts before the next pair's scores need it.
                # chunk c = (q ns0, q ns1, k ns0, k ns1); 6 matmuls per chunk.
                if mt >= 6:
                    return
                if mt == 0:
                    qkt_tiles[pair] = qkt_pool.tile(
                        [128, 2 * N], FP16, tag="qkt", name=f"qkt_{pair}"
                    )
                for j in range(4 * mt, 4 * mt + 4):
                    c, kt = j // 6, j % 6
                    # host-permuted W_qkv: pair p's q|k columns live in the
                    # contiguous slab [256p, 256p+256) = [q(128) | k(128)]
                    col0 = pair * 256 + (0 if c < 2 else 128)
                    ns = c % 2
                    if kt == 0:
                        qkt_state[pair] = _ps_tile([128, 512], "bc")
                    pqc = qkt_state[pair]
                    nc.tensor.matmul(
                        pqc[:],
                        WQKV[:, kt, col0 : col0 + 128],
                        XT[:, kt, ns * 512 : (ns + 1) * 512],
                        start=(kt == 0),
                        stop=(kt == KT - 1 and not has_bias),
                    )
                    if kt == KT - 1:
                        if has_bias:
                            nc.tensor.matmul(
                                pqc[:],
                                BQKV[0:1, col0 : col0 + 128],
                                ONESROW[0:1, ns * 512 : (ns + 1) * 512],
                                start=False,
                                stop=True,
                            )
                        nc.vector.tensor_copy(
                            qkt_tiles[pair][:, c * 512 : (c + 1) * 512], pqc[:]
                        )

            # qkT prelude for pair 0, kt-major: the 4 psum accumulation
            # groups advance together as each x chunk lands, instead of the
            # whole prelude waiting for all of x.  Two [128,1024] psum tiles
            # hold chunk pairs (q ns0|ns1, k ns0|ns1) side by side, and the
            # evacuation is 2 wide casts instead of 4.
            qkt_tiles[0] = qkt_pool.tile([128, 2 * N], FP16, tag="qkt", name="qkt_0")
            QP = [_ps_tile([128, 1024], "sc") for _ in range(2)]
            for kt in range(KT):
                for c in range(4):
                    col0 = 0 if c < 2 else 128
                    ns = c % 2
                    nc.tensor.matmul(
                        QP[c // 2][:, ns * 512 : (ns + 1) * 512],
                        WQKV[:, kt, col0 : col0 + 128],
                        XT[:, kt, ns * 512 : (ns + 1) * 512],
                        start=(kt == 0),
                        stop=(kt == KT - 1 and not has_bias),
                    )
            if has_bias:
                for c in range(4):
                    col0 = 0 if c < 2 else 128
                    ns = c % 2
                    nc.tensor.matmul(
                        QP[c // 2][:, ns * 512 : (ns + 1) * 512],
                        BQKV[0:1, col0 : col0 + 128],
                        ONESROW[0:1, ns * 512 : (ns + 1) * 512],
                        start=False,
                        stop=True,
                    )
            # evacuate the two psum tiles in PARALLEL (DVE + the otherwise
            # idle scalar engine) — serially they'd add ~1.2us to the first
            # exp's critical path
            nc.vector.tensor_copy(qkt_tiles[0][:, 0:1024], QP[0][:])
            nc.scalar.copy(qkt_tiles[0][:, 1024:2048], QP[1][:])

            def do_norm(pend):
                # broadcast 1/sums rows to [128, 512] (head0 -> partitions
                # 0:63, head1 -> 64:127) and normalize ATTNT in place
                ph, IR0, IR1 = pend
                for ns in range(2):
                    nsl = slice(ns * 512, (ns + 1) * 512)
                    BCt = _ps_tile([128, 512], "bc")
                    nc.tensor.matmul(
                        BCt[0:64, :], ONES1x64[:], IR0[0:1, nsl],
                        start=True, stop=True,
                    )
                    nc.tensor.matmul(
                        BCt[64:128, :], ONES1x64[:], IR1[0:1, nsl],
                        start=True, stop=True,
                        tile_position=(0, 64),
                    )
                    nc.vector.tensor_mul(
                        ATTNT[:, ph, nsl], ATTNT[:, ph, nsl], BCt[:]
                    )

            norm_pending = None
            prev = None  # (hp, [EST per mt])
            OUTT = None
            pre_po = []  # proj tiles (0,0)/(0,1) pre-accumulated during it6
            for hp in range(HP + 1):
                if hp < HP:
                    h0, h1 = 2 * hp, 2 * hp + 1
                cur = []
                if prev is not None:
                    ph, pest = prev
                for mt in range(8):
                    # deferred normalize of pair hp-2: its 1/sums rows came
                    # back from the DMA shuffle during iterations 0-1
                    if norm_pending is not None and mt == 6:
                        do_norm(norm_pending)
                        norm_pending = None

                    # final iteration has no scores/qkT filler and stalls on
                    # the est/flush chain — fill the gaps with the first two
                    # proj tiles' ready ct accumulations (pairs 0-3 are
                    # normalized long before; pair 4 right after mt 6's norm)
                    if hp == HP and mt in (0, 2):
                        po = _ps_tile([128, 384], "sc")
                        cs = mt // 2
                        for ct in range(4):
                            nc.tensor.matmul(
                                po[:, 0:384],
                                ATTNT[:, ct, 0:128],
                                WPROJ[:, ct, cs * 384 : (cs + 1) * 384],
                                start=(ct == 0),
                                stop=False,
                            )
                        pre_po.append(po)
                    if hp == HP and mt == 7:
                        for cs in range(2):
                            nc.tensor.matmul(
                                pre_po[cs][:, 0:384],
                                ATTNT[:, 4, 0:128],
                                WPROJ[:, 4, cs * 384 : (cs + 1) * 384],
                                start=False,
                                stop=False,
                            )

                    # next pair's qkT half-chunk: elastic PE filler
                    if hp + 1 < HP:
                        qkt_half(hp + 1, mt)

                    # phase 1 of the current pair (the est = exp * epos DVE
                    # multiply is deferred to the end of the iteration: its
                    # consumer is a whole pair-loop away, and issuing it last
                    # keeps the DVE free for the phase-2 flush copies that
                    # gate the next attn@v accumulation)
                    EPP = ESTP = None
                    if hp < HP:
                        # epos loads stay on the sync sequencer so they queue
                        # BEHIND the critical startup input loads; the bursty
                        # pair-boundary shuffle DMAs go to gpsimd instead
                        # epos + deferred weights all ride the gated gpsimd
                        # queue; the sync queue stays empty mid-loop for the
                        # latency-critical norm shuffle DMAs
                        EPP = epp_pool.tile([128, 2 * N], FP16, tag="epp")
                        nc.sync.dma_start(out=EPP[:, 0:N], in_=epos_ext[h0, mt * 128 : (mt + 1) * 128, :])
                        nc.sync.dma_start(out=EPP[:, N : 2 * N], in_=epos_ext[h1, mt * 128 : (mt + 1) * 128, :])
                        if hp == 0 and mt == 1:
                            # q|k slabs for pairs 2-5: needed from iteration 1
                            # on; slotted into the epos stream so they don't
                            # delay the first est (EPP has 4 tiles of
                            # lookahead to absorb the bubble)
                            nc.sync.dma_start(
                                out=WQKV[:, :, 512:1536], in_=wqkv_r[:, :, 512:1536]
                            )
                        if hp == 1 and mt == 1:
                            # Wproj is only needed by the final projection
                            nc.sync.dma_start(
                                out=WPROJ[:],
                                in_=wproj_ext.rearrange("(t p) n -> p t n", p=128),
                            )

                        # scores for the two heads interleaved (h0,ns0),
                        # (h1,ns0), (h0,ns1), (h1,ns1): consecutive matmuls
                        # use disjoint PE row groups (0:64 vs 64:128, via
                        # base_partition-derived tile_position) and separate
                        # PSUM banks, so the hardware runs each h0/h1 pair
                        # CONCURRENTLY — the 4 matmuls finish in ~2 stream
                        # slots instead of 4.  One combined FD=2048 exp over
                        # the 4-bank pair tile then saves the per-instruction
                        # ACT overhead (~350 cyc) vs two per-head exps.
                        QKTh = qkt_tiles[hp]
                        ESTP = est_pool.tile([128, 2 * N], FP16, tag="est")
                        SCPh = [_ps_tile([128, N], "sc") for _ in range(2)]
                        for ns in range(2):
                            nsl = slice(ns * 512, (ns + 1) * 512)
                            for hh in range(2):
                                row = slice(64 * hh, 64 * hh + 64)
                                nc.tensor.matmul(
                                    SCPh[hh][:, nsl],
                                    QKTh[row, N + mt * 128 : N + (mt + 1) * 128],
                                    QKTh[row, nsl],
                                    start=True, stop=True,
                                )
                        for hh in range(2):
                            nc.scalar.activation(
                                ESTP[:, hh * N : (hh + 1) * N], SCPh[hh][:], EXP,
                                scale=SCALE,
                            )
                        cur.append(ESTP)
                        # phase-breaker: with exactly 2 sc allocations per mt
                        # on the 2-buffer ring, each head's next scores wait
                        # on ITS OWN previous exp (zero slack for whichever
                        # exp finishes last, so the scheduler splits the h0/h1
                        # matmul pairs and the row-group packing is lost).  A
                        # third tiny allocation rotates the ring so both
                        # heads land on fast-completing buffers.
                        SCD = _ps_tile([1, 8], "sc")
                        nc.tensor.matmul(
                            SCD[0:1, 0:1], ONES1x64[0:1, 0:1], WARM[0:1, 0:1],
                            start=True, stop=True,
                        )

                    # phase 2 of the previous pair (issued after the scores so
                    # an OUTT slot-reuse wait never blocks them): head0 of the
                    # pair sweeps est m-tiles 0..7 during mts 0-3, head1
                    # during mts 4-7.  outT_h[0:64] = attn@v rows, outT_h[64]
                    # = softmax denominators (from v_aug's ones column).
                    if prev is not None:
                        h_loc, sub = divmod(mt, 4)
                        hg = 2 * ph + h_loc
                        if sub == 0:
                            OUTT = _ps_tile([65, 1024], "ot")
                        for m2 in (2 * sub, 2 * sub + 1):
                            for ns in range(2):
                                nsl = slice(ns * 512, (ns + 1) * 512)
                                nc.tensor.matmul(
                                    OUTT[:, nsl],
                                    VN65[:, m2, hg * E : hg * E + E],
                                    pest[m2][:, h_loc * N + ns * 512 : h_loc * N + (ns + 1) * 512],
                                    start=(m2 == 0), stop=(m2 == 7),
                                )
                        if sub == 3:
                            # flush this head on DVE (ACT is saturated by the
                            # exp chain; GPSIMD has no PSUM port): rows 0:63
                            # -> ATTNT (head0 directly; head1 staged and
                            # repartitioned to 64:128 by an SBUF->SBUF DMA),
                            # denominator row 64 -> SGP (same partition).
                            nc.vector.tensor_copy(
                                SGP[64:65, h_loc * N : (h_loc + 1) * N],
                                OUTT[64:65, :],
                            )
                            if h_loc == 0:
                                nc.vector.tensor_copy(ATTNT[0:64, ph, :], OUTT[0:64, :])
                            else:
                                STG1 = stg_pool.tile([64, N], FP16, tag="stg")
                                nc.vector.tensor_copy(STG1[:], OUTT[0:64, :])
                                nc.gpsimd.dma_start(
                                    out=ATTNT[64:128, ph, :], in_=STG1[:]
                                )
                                # reshape both denominator rows across 64
                                # lanes, invert, and send 1/sums back as
                                # [1, N] fp16 rows for the broadcast matmuls.
                                # These ride the sync queue (HWDGE, ~0.6us
                                # first-byte, and empty mid-loop) — the whole
                                # chain is norm-latency-critical.
                                S2 = s2_pool.tile([64, 32], F32, tag="s2")
                                for i in range(2):
                                    nc.gpsimd.dma_start(
                                        out=S2[:, 16 * i : 16 * (i + 1)],
                                        in_=SGP[64:65, i * N : (i + 1) * N].rearrange(
                                            "o (p f) -> o p f", p=64
                                        ),
                                    )
                                RI = s2_pool.tile([64, 32], F32, tag="ri")
                                nc.vector.reciprocal(RI[:], S2[:])
                                RI16 = s2_pool.tile([64, 32], FP16, tag="ri16")
                                nc.vector.tensor_copy(RI16[:], RI[:])
                                IR0 = invr_pool.tile([1, N], FP16, tag="invr")
                                IR1 = invr_pool.tile([1, N], FP16, tag="invr")
                                for i, ir in enumerate((IR0, IR1)):
                                    nc.gpsimd.dma_start(
                                        out=ir[0:1, :].rearrange(
                                            "o (p f) -> o p f", p=64
                                        ),
                                        in_=RI16[:, 16 * i : 16 * (i + 1)],
                                    )
                                norm_pending = (ph, IR0, IR1)

                    # deferred phase-1 tail: est = exp(scores) * epos
                    if ESTP is not None:
                        nc.vector.tensor_mul(ESTP[:], ESTP[:], EPP[:])

                    # v-projection filler groups scheduled for this (hp, mt)
                    for nt_, vs_, tag_ in vp_sched.get((hp, mt), ()):
                        vproj_group(nt_, vs_, tag_)

                if hp < HP:
                    prev = (hp, cur)

            # tail: normalize the last pair one n-half at a time, running the
            # projection n-tiles of each half as soon as that half is
            # normalized — the second broadcast hides under the first four
            # projection tiles
            def do_norm_half(pend, ns):
                ph, IR0, IR1 = pend
                nsl = slice(ns * 512, (ns + 1) * 512)
                BCt = _ps_tile([128, 512], "bc")
                nc.tensor.matmul(
                    BCt[0:64, :], ONES1x64[:], IR0[0:1, nsl],
                    start=True, stop=True,
                )
                nc.tensor.matmul(
                    BCt[64:128, :], ONES1x64[:], IR1[0:1, nsl],
                    start=True, stop=True,
                    tile_position=(0, 64),
                )
                nc.vector.tensor_mul(
                    ATTNT[:, ph, nsl], ATTNT[:, ph, nsl], BCt[:]
                )

            # ---- output projection tail ----
            # pre_po (0,0)/(0,1) already hold ct0..4 from iteration 6.  The
            # pair-5 norm waits on the shuffle/reciprocal latency chain; keep
            # the in-order PE queue busy under it with tile (1,0)'s ct0..4,
            # then norm, then stream the ct=5 finishes + remaining tiles.
            def _proj_finish(po, nt, cs, of):
                nc.tensor.matmul(
                    po[:, 0:384],
                    ATTNT[:, KT - 1, nt * 128 : (nt + 1) * 128],
                    WPROJ[:, KT - 1, cs * 384 : (cs + 1) * 384],
                    start=False,
                    stop=(not has_bias),
                )
                if has_bias:
                    nc.tensor.matmul(
                        po[:, 0:384],
                        ONESROW[0:1, nt * 128 : (nt + 1) * 128],
                        BPROJ[0:1, cs * 384 : (cs + 1) * 384],
                        start=False,
                        stop=True,
                    )
                nc.vector.tensor_copy(of[:, cs * 384 : (cs + 1) * 384], po[:, 0:384])

            poC = _ps_tile([128, 384], "ot")
            for ct in range(KT - 1):
                nc.tensor.matmul(
                    poC[:, 0:384],
                    ATTNT[:, ct, 128:256],
                    WPROJ[:, ct, 0:384],
                    start=(ct == 0),
                    stop=False,
                )

            if norm_pending is not None:
                do_norm(norm_pending)
                norm_pending = None

            OF0 = outsb_pool.tile([128, C], F32, tag="of")
            _proj_finish(pre_po[0], 0, 0, OF0)
            _proj_finish(pre_po[1], 0, 1, OF0)
            nc.sync.dma_start(out=out_ext[0:128, :], in_=OF0[:])

            OF1 = outsb_pool.tile([128, C], F32, tag="of")
            _proj_finish(poC, 1, 0, OF1)
            poD = _ps_tile([128, 384], "bc")
            for ct in range(KT - 1):
                nc.tensor.matmul(
                    poD[:, 0:384],
                    ATTNT[:, ct, 128:256],
                    WPROJ[:, ct, 384:768],
                    start=(ct == 0),
                    stop=False,
                )
            _proj_finish(poD, 1, 1, OF1)
            nc.sync.dma_start(out=out_ext[128:256, :], in_=OF1[:])

            for nt in range(2, NT):
                OF = outsb_pool.tile([128, C], F32, tag="of")
                for cs in range(2):
                    po = _ps_tile([128, 384], _vtags[(nt * 2 + cs) % 4])
                    dst = po[:, 0:384]
                    for ct in range(KT):
                        nc.tensor.matmul(
                            dst,
                            ATTNT[:, ct, nt * 128 : (nt + 1) * 128],
                            WPROJ[:, ct, cs * 384 : (cs + 1) * 384],
                            start=(ct == 0),
                            stop=(ct == KT - 1 and not has_bias),
                        )
                    if has_bias:
                        nc.tensor.matmul(
                            dst,
                            ONESROW[0:1, nt * 128 : (nt + 1) * 128],
                            BPROJ[0:1, cs * 384 : (cs + 1) * 384],
                            start=False,
                            stop=True,
                        )
                    nc.vector.tensor_copy(OF[:, cs * 384 : (cs + 1) * 384], dst)
                    if nt == NT - 1:
                        # last tile: ship each half as soon as it's ready so
                        # the final DMA drain starts ~1.7us earlier
                        nc.sync.dma_start(
                            out=out_ext[nt * 128 : (nt + 1) * 128, cs * 384 : (cs + 1) * 384],
                            in_=OF[:, cs * 384 : (cs + 1) * 384],
                        )
                if nt != NT - 1:
                    nc.sync.dma_start(out=out_ext[nt * 128 : (nt + 1) * 128, :], in_=OF[:])

    _split_excess_waits(nc)
    return nc


_BUILT = {}


def _get_nc(has_bias):
    if has_bias not in _BUILT:
        _BUILT[has_bias] = build(has_bias)
    return _BUILT[has_bias]


_QKV_COL_PERM = np.concatenate(
    [
        np.concatenate([np.arange(p * 128, (p + 1) * 128),
                        768 + np.arange(p * 128, (p + 1) * 128)])
        for p in range(HP)
    ]
    + [1536 + np.arange(768)]
)


def prepare_inputs(x, pos_embedding, W_qkv, b_qkv, W_proj, b_proj):
    B = x.shape[0]
    has_bias = bool(np.any(b_qkv)) or bool(np.any(b_proj))
    # permute q|k columns pair-contiguously (see kernel comment): block p is
    # [q cols 128p:128p+128 | k cols 768+128p:...]; the v block is unchanged
    wqkv16 = np.ascontiguousarray(
        np.asarray(W_qkv).astype(np.float16)[:, _QKV_COL_PERM]
    )
    wproj16 = np.ascontiguousarray(W_proj).astype(np.float16)
    epos16 = np.exp(
        pos_embedding[0].transpose(0, 2, 1).astype(np.float32) * SCALE
    ).astype(np.float16)
    epos16 = np.ascontiguousarray(epos16)
    in_maps = []
    for b in range(B):
        m = {
            "xt": np.ascontiguousarray(x[b].T).astype(np.float16),
            "wqkv": wqkv16,
            "wproj": wproj16,
            "epos": epos16,
        }
        if has_bias:
            m["bqkv"] = (
                b_qkv.reshape(-1)[_QKV_COL_PERM].reshape(1, -1).astype(np.float16)
            )
            m["bproj"] = b_proj.reshape(1, -1).astype(np.float16)
        in_maps.append(m)
    return has_bias, in_maps


def kernel(x, pos_embedding, W_qkv, b_qkv, W_proj, b_proj):
    from concourse.bass_utils import run_bass_kernel_spmd

    x = np.asarray(x)
    pos_embedding = np.asarray(pos_embedding)
    W_qkv = np.asarray(W_qkv)
    b_qkv = np.asarray(b_qkv)
    W_proj = np.asarray(W_proj)
    b_proj = np.asarray(b_proj)

    has_bias, in_maps = prepare_inputs(x, pos_embedding, W_qkv, b_qkv, W_proj, b_proj)
    nc = _get_nc(has_bias)
    res = run_bass_kernel_spmd(nc, in_maps, list(range(N_CORES)), trace=False)
    out = np.stack([res.results[i]["out"] for i in range(N_CORES)], axis=0)
    return out.astype(np.float32)

